# revision 54
# baseline (speedup 1.0000x reference)
"""Trainium2 Bass kernel for nn_Attention_45578192945380.

Full (unsharded) inputs -> full output. Sharding: core c handles batch b=c//2
and head group g=c%2 (heads 4g..4g+4). Zero cross-core communication; the two
cores sharing a batch produce partial out-projections that are summed on host.

Math restructuring (validated vs reference in fp64/fp32):
  - softmax_j(s_ij + B*bias_i + B*bias_j) == softmax_j(s_ij + B*bias_j): the
    row term is constant per row and cancels.
  - scores are computed TRANSPOSED (sT[j,i] = k_j . q_i) so B*bias_j is a
    per-partition scalar and folds into the exp ACTIVATE's free bias input,
    and so the AV matmul (lhsT = [v | 1]) needs no transposes.
  - the appended ones-column of V makes column 64 of the AV output the softmax
    denominator; normalization is deferred to after AV (divide, then project).
  - unnormalized softmax (no max subtraction) is safe here: |scores| <~ 10.
  - rstd = exp(-0.5*ln(var+eps)) keeps every ACT call in one table set
    (natural_log_exp) together with the softmax exp.
"""

import os
import sys
from contextlib import ExitStack

import numpy as np

for _p in ("/opt/trn_rl_repo", "/root/.axon_site/_ro/trn_rl_repo"):
    if os.path.isdir(_p) and _p not in sys.path:
        sys.path.insert(0, _p)

import ml_dtypes

import concourse.bass as bass
import concourse.bacc as bacc
import concourse.tile as tile
from concourse import mybir
from concourse.bass_utils import run_bass_kernel_spmd
from concourse.masks import make_identity

F32 = mybir.dt.float32
BF16 = mybir.dt.bfloat16
AF = mybir.ActivationFunctionType
OP = mybir.AluOpType
BFNP = ml_dtypes.bfloat16

B, N, DIM = 4, 2048, 512
HEADS, DH = 8, 64
EPS = 1e-5
NT = N // 128          # 16 n-chunks of 128
DC = DIM // 128        # 4 d-chunks
SCALE = DH ** -0.5     # 0.125
NCORES = 8


def _emit(tc: tile.TileContext, ctx: ExitStack, aps: dict, affine: bool):
    nc = tc.nc

    const = ctx.enter_context(tc.tile_pool(name="const", bufs=1))
    big = ctx.enter_context(tc.tile_pool(name="big", bufs=1))

    # ---- constants / weights ----
    # weights + pose bias ride the scalar HWDGE queue so the x loads own the
    # sync queue from t=0
    wq_sb = const.tile([128, DC, 256], BF16)
    wk_sb = const.tile([128, DC, 256], BF16)
    wv_sb = const.tile([128, DC, 256], BF16)
    wo_sb = const.tile([128, 2, 512], BF16)
    for dc in range(DC):
        nc.gpsimd.dma_start(out=wq_sb[:, dc, :], in_=aps["wq"][dc * 128:(dc + 1) * 128, :])
        nc.gpsimd.dma_start(out=wk_sb[:, dc, :], in_=aps["wk"][dc * 128:(dc + 1) * 128, :])
        nc.gpsimd.dma_start(out=wv_sb[:, dc, :], in_=aps["wv"][dc * 128:(dc + 1) * 128, :])
    for cc in range(2):
        nc.gpsimd.dma_start(out=wo_sb[:, cc, :], in_=aps["wo"][cc * 128:(cc + 1) * 128, :])
    pb_sb = const.tile([128, NT], F32)
    nc.gpsimd.dma_start(out=pb_sb[:, :], in_=aps["pb"].rearrange("(t p) -> p t", p=128))

    gam_bc = bet_bc = None
    if affine:
        gam_bc = const.tile([128, DIM], F32)
        bet_bc = const.tile([128, DIM], F32)
        ga, be = aps["gam"], aps["bet"]
        nc.sync.dma_start(
            out=gam_bc[:, :],
            in_=bass.AP(tensor=ga.tensor, offset=ga.offset, ap=[[0, 128]] + list(ga.ap)),
        )
        nc.sync.dma_start(
            out=bet_bc[:, :],
            in_=bass.AP(tensor=be.tensor, offset=be.offset, ap=[[0, 128]] + list(be.ap)),
        )

    # ---- persistent activations ----
    x_sb = big.tile([128, NT, DIM], F32)         # 32 KiB/part
    xnT = big.tile([128, DC, N], BF16)           # 16 KiB/part
    qT = big.tile([128, 2, N], BF16)             # 8 KiB/part
    kT = big.tile([128, 2, N], BF16)
    v_sb = big.tile([128, NT, 4, DH + 1], BF16)  # [j-chunk, head, v|1]
    aoT = big.tile([128, 2, N], BF16)            # normalized attn out, transposed
    stats = const.tile([128, NT, 2], F32)        # (mean, var) per n-chunk
    lnv = const.tile([128, NT], F32)
    rstd = const.tile([128, NT], F32)
    negmr = const.tile([128, NT], F32)
    eps_sb = const.tile([128, 1], F32)
    nc.vector.memset(eps_sb, EPS)
    zero_sb = const.tile([128, 1], F32)
    nc.vector.memset(zero_sb, 0.0)
    ident = const.tile([128, 128], BF16)
    make_identity(nc, ident)
    ones64 = const.tile([DH + 1, DH], F32)
    nc.vector.memset(ones64, 1.0)

    # ================= phase 1: LayerNorm + transpose =================
    with tc.tile_pool(name="ph1", bufs=4) as ph1, \
         tc.tile_pool(name="tps", bufs=4, space="PSUM") as tps:
        for t in range(NT):
            nc.sync.dma_start(out=x_sb[:, t, :], in_=aps["x"][t * 128:(t + 1) * 128, :])

        # Phase 2 (QKV) pieces are emitted as soon as the four n-chunks they
        # read are transposed, keeping the in-order PE queue dense.
        # PSUM: tps 4 banks + qk 2 + v 2 = 8
        with tc.tile_pool(name="qkvps", bufs=2, space="PSUM") as qkvps:
            def qk_piece(w_sb, dst, cc, p):
                ps = qkvps.tile([128, 512], F32, tag="qk", name=f"qk_{cc}_{p}")
                for dc in range(DC):
                    nc.tensor.matmul(
                        ps, w_sb[:, dc, cc * 128:(cc + 1) * 128],
                        xnT[:, dc, p * 512:(p + 1) * 512],
                        start=(dc == 0), stop=(dc == DC - 1),
                    )
                nc.vector.tensor_copy(out=dst[:, cc, p * 512:(p + 1) * 512], in_=ps)

            def v_chunk(t):
                ps = qkvps.tile([128, 256], F32, tag="v", name=f"v_{t}")
                for dc in range(DC):
                    nc.tensor.matmul(
                        ps, xnT[:, dc, t * 128:(t + 1) * 128], wv_sb[:, dc, :],
                        start=(dc == 0), stop=(dc == DC - 1),
                    )
                for h in range(4):
                    nc.vector.tensor_copy(out=v_sb[:, t, h, 0:DH], in_=ps[:, h * DH:(h + 1) * DH])

            nc.gpsimd.memset(v_sb[:, :, :, DH:DH + 1], 1.0)
            # rstd = exp(-0.5 * ln(var + eps)) ; negmr = -mean * rstd.
            # Group 0's stats are finalized right after its 4 bn passes so the
            # first QKV piece (and the exp stream) starts ~8us earlier; the
            # remaining 12 tiles share one flat pass (keeping the PE emission
            # pattern that stays HAM-warm).
            def finalize_stats(sl):
                nc.scalar.activation(out=lnv[:, sl], in_=stats[:, sl, 1], func=AF.Ln, bias=eps_sb[:, :])
                nc.scalar.activation(out=rstd[:, sl], in_=lnv[:, sl], func=AF.Exp, bias=zero_sb[:, :], scale=-0.5)
                nc.vector.tensor_scalar(out=stats[:, sl, 0], in0=stats[:, sl, 0], scalar1=-1.0, scalar2=None, op0=OP.mult)
                nc.vector.tensor_tensor_scan(out=negmr[:, sl], data0=stats[:, sl, 0], data1=rstd[:, sl],
                                             initial=0.0, op0=OP.bypass, op1=OP.mult)

            for t in range(NT):
                st6 = ph1.tile([128, 6], F32, tag="bnst")
                nc.vector.bn_stats(out=st6, in_=x_sb[:, t, :])
                nc.vector.bn_aggr(out=stats[:, t, :], in_=st6)
                if t == 3:
                    finalize_stats(slice(0, 4))
            finalize_stats(slice(4, NT))
            for g in range(4):
                for t in range(4 * g, 4 * g + 4):
                    if affine:
                        xh = ph1.tile([128, DIM], F32, tag="xh")
                        nc.vector.tensor_scalar(
                            out=xh, in0=x_sb[:, t, :],
                            scalar1=rstd[:, t:t + 1], scalar2=negmr[:, t:t + 1],
                            op0=OP.mult, op1=OP.add,
                        )
                        xg = ph1.tile([128, DIM], F32, tag="xg")
                        nc.vector.tensor_tensor(out=xg, in0=xh, in1=gam_bc[:, :], op=OP.mult)
                        xn_t = ph1.tile([128, DIM], BF16, tag="xn")
                        nc.vector.tensor_tensor(out=xn_t, in0=xg, in1=bet_bc[:, :], op=OP.add)
                    else:
                        # LN apply on DVE (2x_2P tensor_scalar); ScalarE keeps
                        # the 4 transpose evacuations per tile, balancing the
                        # ramp's two busiest engines at ~1.2us/tile each
                        xn_t = ph1.tile([128, DIM], BF16, tag="xn")
                        nc.vector.tensor_scalar(
                            out=xn_t, in0=x_sb[:, t, :],
                            scalar1=rstd[:, t:t + 1], scalar2=negmr[:, t:t + 1],
                            op0=OP.mult, op1=OP.add,
                        )
                    for dc in range(DC):
                        # transpose as a REGULAR matmul (xn.T @ I): unlike
                        # transpose-mode, it counts as PE-busy for the HAM
                        # clock gate, so the PE is warm by the time QKV starts
                        tp = tps.tile([128, 128], F32, tag="tp")
                        nc.tensor.matmul(tp, xn_t[:, dc * 128:(dc + 1) * 128],
                                         ident, start=True, stop=True)
                        # evacuate on ScalarE: it is idle during the ramp and
                        # the DVE is the ramp's critical engine
                        nc.scalar.copy(out=xnT[:, dc, t * 128:(t + 1) * 128], in_=tp)
                qk_piece(wq_sb, qT, 0, g)
                qk_piece(wk_sb, kT, 0, g)
                for tv in range(g * 4, g * 4 + 4):
                    v_chunk(tv)
                qk_piece(wq_sb, qT, 1, g)
                qk_piece(wk_sb, kT, 1, g)

    # ================= phase 3: attention per head =================
    with tc.tile_pool(name="dotsps", bufs=2, space="PSUM") as dots_pool, \
         tc.tile_pool(name="avps", bufs=2, space="PSUM") as av_pool, \
         tc.tile_pool(name="epool", bufs=8) as epool, \
         tc.tile_pool(name="avsb", bufs=4) as avsb_pool, \
         tc.tile_pool(name="rbc", bufs=4) as rbc_pool, \
         tc.tile_pool(name="ostage", bufs=4) as ostage, \
         tc.tile_pool(name="dramb", bufs=2, space="DRAM") as dram_pool:
        for h in range(4):
            cc, off = h // 2, 64 * (h % 2)
            av_ps = [av_pool.tile([DH + 1, 1024], F32, tag="av", name=f"avps_h{h}_{i}")
                     for i in range(2)]
            # Software-pipelined: AV matmuls for chunk J-1 are emitted after
            # the dots matmuls for chunk J, so the in-order PE queue never
            # stalls waiting for exp(J) (head-of-line blocking keeps the PE
            # dense and the HAM clock warm).
            pend = []
            for J in range(NT):
                eTs = []
                for ihalf in range(2):
                    dps = dots_pool.tile([128, 1024], F32, tag="dots")
                    for p in range(2):
                        nc.tensor.matmul(
                            dps[:, p * 512:(p + 1) * 512],
                            kT[off:off + 64, cc, J * 128:(J + 1) * 128],
                            qT[off:off + 64, cc, ihalf * 1024 + p * 512: ihalf * 1024 + (p + 1) * 512],
                            start=True, stop=True,
                        )
                    eT = epool.tile([128, 1024], BF16, tag="e")
                    nc.scalar.activation(out=eT, in_=dps, func=AF.Exp,
                                         bias=pb_sb[:, J:J + 1], scale=SCALE)
                    eTs.append(eT)

                def emit_av(Jp, eTp):
                    for ihalf in range(2):
                        for p in range(2):
                            nc.tensor.matmul(
                                av_ps[ihalf][:, p * 512:(p + 1) * 512],
                                v_sb[:, Jp, h, :], eTp[ihalf][:, p * 512:(p + 1) * 512],
                                start=(Jp == 0), stop=(Jp == NT - 1),
                            )

                if pend:
                    emit_av(*pend.pop())
                pend.append((J, eTs))
            emit_av(*pend.pop())
            for ihalf in range(2):
                av_sb = avsb_pool.tile([DH + 1, 1024], F32, tag="avsb")
                # For the last head the two i-half tails are parallelized
                # across engines/queues (the exp stream is finished, so the
                # scalar engine and its DMA queue are free).
                last = (h == 3)
                if last and ihalf == 1:
                    nc.scalar.copy(out=av_sb, in_=av_ps[ihalf])
                else:
                    nc.vector.tensor_copy(out=av_sb, in_=av_ps[ihalf])
                deng = nc.scalar if (last and ihalf == 1) else nc.sync
                # broadcast the denominator row to 64 partitions via a DRAM
                # bounce (partition-step-0 read), then reciprocal on DVE
                dbuf = dram_pool.tile([1, 1024], F32, tag="den")
                deng.dma_start(out=dbuf[:, :], in_=av_sb[DH:DH + 1, :])
                d_bc = rbc_pool.tile([64, 1024], F32, tag="dbc")
                da = dbuf[:, :]
                deng.dma_start(
                    out=d_bc,
                    in_=bass.AP(tensor=da.tensor, offset=da.offset,
                                ap=[[0, 64]] + list(da.ap[1:])),
                )
                r_bc = rbc_pool.tile([64, 1024], F32, tag="rbc")
                nc.vector.reciprocal_approx_fast(out=r_bc, in_=d_bc)
                nc.vector.tensor_tensor(
                    out=aoT[off:off + 64, cc, ihalf * 1024:(ihalf + 1) * 1024],
                    in0=av_sb[0:DH, :], in1=r_bc, op=OP.mult,
                )
                if h == 3:
                    # output projection for this i-half; PSUM comes from the
                    # (now idle) dots pool slots
                    for p in (2 * ihalf, 2 * ihalf + 1):
                        for mc in range(4):
                            po = dots_pool.tile([128, 1024], F32, tag="dots",
                                                name=f"op_{p}_{mc}")
                            for ccx in range(2):
                                nc.tensor.matmul(
                                    po[:, 0:512], wo_sb[:, ccx, mc * 128:(mc + 1) * 128],
                                    aoT[:, ccx, p * 512:(p + 1) * 512],
                                    start=(ccx == 0), stop=(ccx == 1),
                                )
                            st = ostage.tile([128, 512], F32, tag="ost")
                            # split tail evacuations across the two engines
                            # that can read PSUM (ACT is idle after last exp)
                            if mc % 2 == 0:
                                nc.scalar.copy(out=st, in_=po[:, 0:512])
                            else:
                                nc.vector.tensor_copy(out=st, in_=po[:, 0:512])
                            oeng = nc.sync if mc % 2 == 0 else nc.scalar
                            oeng.dma_start(
                                out=aps["out"][mc * 128:(mc + 1) * 128, p * 512:(p + 1) * 512],
                                in_=st,
                            )


_CACHE: dict = {}


def _build(affine: bool):
    key = ("nc", affine)
    if key in _CACHE:
        return _CACHE[key]
    nc = bacc.Bacc("TRN2", target_bir_lowering=False, debug=False,
                   num_devices=NCORES)
    aps = {
        "x": nc.dram_tensor("x", [N, DIM], F32, kind="ExternalInput").ap(),
        "pb": nc.dram_tensor("pb", [N], F32, kind="ExternalInput").ap(),
        "wq": nc.dram_tensor("wq", [DIM, 256], BF16, kind="ExternalInput").ap(),
        "wk": nc.dram_tensor("wk", [DIM, 256], BF16, kind="ExternalInput").ap(),
        "wv": nc.dram_tensor("wv", [DIM, 256], BF16, kind="ExternalInput").ap(),
        "wo": nc.dram_tensor("wo", [256, DIM], BF16, kind="ExternalInput").ap(),
        "out": nc.dram_tensor("out", [DIM, N], F32, kind="ExternalOutput").ap(),
    }
    if affine:
        aps["gam"] = nc.dram_tensor("gam", [DIM], F32, kind="ExternalInput").ap()
        aps["bet"] = nc.dram_tensor("bet", [DIM], F32, kind="ExternalInput").ap()
    with tile.TileContext(nc) as tc:
        with ExitStack() as ctx:
            _emit(tc, ctx, aps, affine)
    nc.compile()
    _CACHE[key] = nc
    return nc


def _prep_in_maps(x, pose_bias, ln_gamma, ln_beta, w_qkv, w_out, beta):
    x = np.asarray(x, np.float32)
    pose = np.asarray(pose_bias, np.float32)
    gam = np.asarray(ln_gamma, np.float32)
    bet = np.asarray(ln_beta, np.float32)
    wqkv = np.asarray(w_qkv, np.float32)
    wo = np.asarray(w_out, np.float32)
    bval = float(np.asarray(beta))
    affine = not (np.all(gam == 1.0) and np.all(bet == 0.0))
    in_maps = []
    for c in range(NCORES):
        b, g = c // 2, c % 2
        sl = slice(g * 256, (g + 1) * 256)
        m = {
            "x": np.ascontiguousarray(x[b]),
            "pb": np.ascontiguousarray(bval * pose[b]),
            "wq": np.ascontiguousarray(wqkv[:, 0:512][:, sl]).astype(BFNP),
            "wk": np.ascontiguousarray(wqkv[:, 512:1024][:, sl]).astype(BFNP),
            "wv": np.ascontiguousarray(wqkv[:, 1024:1536][:, sl]).astype(BFNP),
            "wo": np.ascontiguousarray(wo[sl, :]).astype(BFNP),
        }
        if affine:
            m["gam"] = gam
            m["bet"] = bet
        in_maps.append(m)
    return in_maps, affine


def _gather(results):
    outs = []
    for b in range(B):
        o = results[2 * b]["out"].astype(np.float32) + results[2 * b + 1]["out"].astype(np.float32)
        outs.append(o.T)
    return np.ascontiguousarray(np.stack(outs))


def _ensure_ntff_shim():
    """This image's antenv lacks axon_hooks; register the NTFF profile hook
    ourselves so run_bass_kernel_spmd(trace=True) can capture exec time."""
    import types
    if "antenv.axon_hooks" in sys.modules:
        return
    mod = types.ModuleType("antenv.axon_hooks")
    state = {"hook": None}
    mod.set_axon_ntff_profile_hook = lambda h: state.__setitem__("hook", h)
    mod.get_axon_ntff_profile_hook = lambda: state["hook"]
    sys.modules["antenv.axon_hooks"] = mod
    try:
        from trn_agent_boot.trn_boot import _ntff_profile_via_ctypes
        mod.set_axon_ntff_profile_hook(
            _ntff_profile_via_ctypes("/opt/axon/libaxon_pjrt.so"))
    except Exception:
        pass


def run(trace=False, **inputs):
    if trace:
        _ensure_ntff_shim()
    in_maps, affine = _prep_in_maps(**inputs)
    nc = _build(affine)
    res = run_bass_kernel_spmd(nc, in_maps, core_ids=list(range(NCORES)),
                               trace=trace)
    return _gather(res.results), res


def kernel(**inputs) -> np.ndarray:
    out, _ = run(trace=False, **inputs)
    return out
